# revision 14
# baseline (speedup 1.0000x reference)
"""Causal self-attention Trainium2 kernel (8 NeuronCores, SPMD), fp8 DoubleRow.

Problem (hardcoded): x [4, 2048, 2048] f32, W_qkv [6144, 2048], W_out [2048, 2048],
16 heads x 128 dim, causal softmax attention + output projection.

Sharding: core c = 2*b + g handles batch b (4) and head-group g (2 groups of 8
heads).  Host sums the two partial out-projections per batch element.

Precision scheme (validated vs reference, rel err ~1.0e-2 on CPU emulation):
- All projections run as fp8e4 (e4m3) DoubleRow matmuls (0.5 cycles/row):
  x quantized e4m3, weights quantized e4m3 after x512 scaling.
- Attention S/AV/rowsum in fp8 (AV+rowsum DoubleRow over k-chunk pairs).
- Early rows are error-amplified (few-key softmax), so the first q-block
  (t<512) runs a bf16 attention path: S from bf16 Q/K, bf16 P, bf16 V
  (V for k<128 additionally recomputed via a bf16 x@Wv projection), and the
  first 128 output rows go through a bf16 output projection.

Scales: qt8/kt8/v8 = psum * 4/512 (=4Q etc.), exp scale = (1/sqrt(128))/16,
rowsum ones = 1/16 so po*recip = 64*O, out-proj psum = 64*O*512*Wo = 32768*out.

No DRAM intermediates: Q/K/V/O live in SBUF across the fused pipeline.
"""

import math

import numpy as np

B = 4
T = 2048
C = 2048
H = 16          # total heads
HG = 8          # heads per core (tensor-parallel group)
D = 128         # head dim
P = 128         # partitions
NCS = C // P    # 16 contraction subtiles
NPR = NCS // 2  # 8 contraction pair-tiles
NTC = T // P    # 16 T chunks of 128
NTB = T // 512  # 4 T blocks of 512
SCALE = 1.0 / math.sqrt(D)
EXP_SCALE = SCALE / 16.0
QSC = 4.0 / 512.0       # psum -> qt8/kt8/v8 scale
OSC = 2.0 ** -15        # out-proj psum -> f32 out

_CACHED = None


def _build(phases="abc", repeat=1, vmode=None):
    import concourse.mybir as mybir
    from concourse import bacc
    from concourse.tile import TileContext

    f32 = mybir.dt.float32
    f32r = mybir.dt.float32r
    f8 = mybir.dt.float8e4
    bf16 = mybir.dt.bfloat16
    EXP = mybir.ActivationFunctionType.Exp
    COPY = mybir.ActivationFunctionType.Copy
    MULT = mybir.AluOpType.mult
    DR = mybir.MatmulPerfMode.DoubleRow

    nc = bacc.Bacc("TRN2", target_bir_lowering=False)

    xt_d = nc.dram_tensor("xt", [NPR, P, 2, T], f8, kind="ExternalInput")
    xt16_d = nc.dram_tensor("xt16", [P, NCS, P], bf16, kind="ExternalInput")
    wq_d = nc.dram_tensor("wq", [HG, P, NCS, D], f8, kind="ExternalInput")
    wk_d = nc.dram_tensor("wk", [HG, P, NCS, D], f8, kind="ExternalInput")
    wv_d = nc.dram_tensor("wv", [P, NCS, HG * D], f8, kind="ExternalInput")
    wv16_d = nc.dram_tensor("wv16", [P, NCS, HG * D], bf16, kind="ExternalInput")
    wo_d = nc.dram_tensor("wo", [P, HG, C], f8, kind="ExternalInput")
    wo16_d = nc.dram_tensor("wo16", [P, HG, C], bf16, kind="ExternalInput")
    tri8_d = nc.dram_tensor("tri8", [P, P], f8, kind="ExternalInput")
    tri16_d = nc.dram_tensor("tri16", [P, P], bf16, kind="ExternalInput")
    ones8_d = nc.dram_tensor("ones8", [P, 2, P], f8, kind="ExternalInput")
    ones16_d = nc.dram_tensor("ones16", [P, P], bf16, kind="ExternalInput")
    out_d = nc.dram_tensor("out", [T, C], f32, kind="ExternalOutput")

    with TileContext(nc) as tc:
        with tc.tile_pool(name="persist", bufs=1) as persist:
            # constants
            tri8_t = persist.tile([P, P], f8, tag="tri8")
            nc.sync.dma_start(tri8_t, tri8_d[:])
            tri16_t = persist.tile([P, P], bf16, tag="tri16")
            nc.sync.dma_start(tri16_t, tri16_d[:])
            ones8_t = persist.tile([P, 2, P], f8, tag="ones8")
            nc.sync.dma_start(ones8_t, ones8_d[:])
            ones16_t = persist.tile([P, P], bf16, tag="ones16")
            nc.sync.dma_start(ones16_t, ones16_d[:])
            # x^T fp8 pair tiles + bf16 first 128 cols
            xt16_t = persist.tile([P, NCS, P], bf16, tag="xt16")
            nc.sync.dma_start(xt16_t, xt16_d[:])
            xt = []
            for pr in range(NPR):
                t_ = persist.tile([P, 2, T], f8, tag=f"xt{pr}")
                nc.sync.dma_start(t_, xt_d[pr])
                xt.append(t_)
            # resident weights
            wv_t = persist.tile([P, NCS, HG * D], f8, tag="wv")
            nc.sync.dma_start(wv_t, wv_d[:])
            wo_t = persist.tile([P, HG, C], f8, tag="wo")
            nc.sync.dma_start(wo_t, wo_d[:])
            # V for all heads (AV lhsT), fp8 + bf16 early chunks
            v8_t = persist.tile([P, NTC, HG, D], f8, tag="v8")
            v16_t = persist.tile([P, 4, HG, D], bf16, tag="v16")
            # output activations
            ot8_t = persist.tile([P, HG, T], f8, tag="ot8")
            ot16_t = persist.tile([P, HG, P], bf16, tag="ot16")

            for _rep in range(repeat):
                with tc.tile_pool(name="psA", bufs=2, space="PSUM") as psA, \
                     tc.tile_pool(name="psO", bufs=2, space="PSUM") as psO, \
                     tc.tile_pool(name="hw", bufs=2) as hwp, \
                     tc.tile_pool(name="hqk", bufs=2) as hqk, \
                     tc.tile_pool(name="hq16", bufs=2) as hq16, \
                     tc.tile_pool(name="pt8", bufs=10) as pt8p, \
                     tc.tile_pool(name="pt16", bufs=3) as pt16p, \
                     tc.tile_pool(name="rc", bufs=4) as rcp:

                    # ---------------- V projection (all heads) ----------------
                    with tc.tile_pool(name="wv16p", bufs=1) as wv16p, \
                         tc.tile_pool(name="psV", bufs=2, space="PSUM") as psV:
                        wv16_t = wv16p.tile([P, NCS, HG * D], bf16, tag="wv16")
                        nc.sync.dma_start(wv16_t, wv16_d[:])
                        for tch in range(NTC):
                            ps = psV.tile([P, 2, 512], f32, tag="psv")
                            for half in range(2):
                                if tch == 0:
                                    # bf16 V proj for k<128 (accuracy)
                                    for cs in range(NCS):
                                        nc.tensor.matmul(
                                            ps[:, half],
                                            xt16_t[:, cs],
                                            wv16_t[:, cs,
                                                   half * 512:(half + 1) * 512],
                                            start=(cs == 0), stop=(cs == NCS - 1))
                                else:
                                    for pr in range(NPR):
                                        nc.tensor.matmul(
                                            ps[:, half],
                                            xt[pr][:, :, tch * P:(tch + 1) * P],
                                            wv_t[:, 2 * pr:2 * pr + 2,
                                                 half * 512:(half + 1) * 512],
                                            start=(pr == 0), stop=(pr == NPR - 1),
                                            perf_mode=DR)
                            with nc.allow_low_precision("fp8 quant"):
                                nc.vector.tensor_scalar_mul(
                                    v8_t[:, tch], ps, QSC)
                                if tch < 4:
                                    nc.vector.tensor_scalar_mul(
                                        v16_t[:, tch], ps, QSC)

                    # ------- head pipeline: attn(h) interleaved with proj(h+1)
                    def load_head(h):
                        wq_t = hwp.tile([P, NCS, D], f8, tag="wq", name="wq_t")
                        nc.sync.dma_start(wq_t, wq_d[h])
                        wk_t = hwp.tile([P, NCS, D], f8, tag="wk", name="wk_t")
                        nc.sync.dma_start(wk_t, wk_d[h])
                        qt_t = hqk.tile([P, T], f8, tag="qt", name="qt_t")
                        kt_t = hqk.tile([P, T], f8, tag="kt", name="kt_t")
                        q16_t = hq16.tile([P, 512], bf16, tag="q16", name="q16_t")
                        k16_t = hq16.tile([P, 512], bf16, tag="k16", name="k16_t")
                        return (wq_t, wk_t, qt_t, kt_t, q16_t, k16_t)

                    def proj_steps(tiles):
                        wq_t, wk_t, qt_t, kt_t, q16_t, k16_t = tiles
                        steps = []
                        for w_t, dst8, dst16 in ((wq_t, qt_t, q16_t),
                                                 (wk_t, kt_t, k16_t)):
                            for tbp in range(2):
                                def step(w_t=w_t, dst8=dst8, dst16=dst16,
                                         tbp=tbp):
                                    tb0, tb1 = 2 * tbp, 2 * tbp + 1
                                    psa = psA.tile([P, 512], f32, tag="psa",
                                                   name="ps_a0")
                                    psb = psA.tile([P, 512], f32, tag="psa",
                                                   name="ps_a1")
                                    # both tb blocks share each stationary
                                    for pr in range(NPR):
                                        for ps_, tb in ((psa, tb0),
                                                        (psb, tb1)):
                                            nc.tensor.matmul(
                                                ps_,
                                                w_t[:, 2 * pr:2 * pr + 2],
                                                xt[pr][:, :, tb * 512:
                                                       (tb + 1) * 512],
                                                start=(pr == 0),
                                                stop=(pr == NPR - 1),
                                                perf_mode=DR)
                                    with nc.allow_low_precision("fp8 quant"):
                                        for ps_, tb in ((psa, tb0),
                                                        (psb, tb1)):
                                            nc.vector.tensor_scalar_mul(
                                                dst8[:, tb * 512:
                                                     (tb + 1) * 512],
                                                ps_, QSC)
                                        if tbp == 0:
                                            nc.vector.tensor_scalar_mul(
                                                dst16, psa, QSC)
                                steps.append(step)
                        return steps

                    def attn_steps(h, tiles):
                        wq_t, wk_t, qt_t, kt_t, q16_t, k16_t = tiles
                        st = {}
                        steps = []

                        def new_block():
                            st["po"] = psO.tile([P, 512], f32, tag="po", name="po_t")
                            st["rs"] = psO.tile([P, 512], f32, tag="po", name="rs_t")
                            st["pts"] = {}

                        def s16_step(m):
                            q0 = m * P
                            if m % 2 == 0:
                                st["ps16"] = psSp.tile([P, 2, 512], f32, tag="pss", name="ps16_t")
                            ps = st["ps16"]
                            nc.tensor.matmul(
                                ps[:, m % 2, q0:], k16_t[:, m * P:(m + 1) * P],
                                q16_t[:, q0:], start=True, stop=True)
                            if m % 2 == 1:
                                pt = pt16p.tile([P, 2, 512], bf16, tag="pt16", name="pt16_t")
                                for j in (m - 1, m):
                                    jq0 = j * P
                                    nc.scalar.activation(
                                        pt[:, j % 2, jq0:], ps[:, j % 2, jq0:],
                                        EXP, scale=EXP_SCALE)
                                    with nc.allow_low_precision("mask"):
                                        nc.gpsimd.tensor_tensor(
                                            pt[:, j % 2, jq0:jq0 + P],
                                            pt[:, j % 2, jq0:jq0 + P],
                                            tri16_t, MULT)
                                    st["pts"][j] = pt

                        def av16_step(m):
                            q0 = m * P
                            pt = st["pts"].pop(m)
                            nc.tensor.matmul(
                                st["po"][:, q0:], v16_t[:, m, h],
                                pt[:, m % 2, q0:],
                                start=(m == 0), stop=(m == 3))
                            nc.tensor.matmul(
                                st["rs"][:, q0:], ones16_t, pt[:, m % 2, q0:],
                                start=(m == 0), stop=(m == 3))

                        def s8_step(kp, jb):
                            ks0 = 2 * kp
                            diag = ks0 - 4 * jb
                            ps = psSp.tile([P, 2, 512], f32, tag="pss", name="ps8_t")
                            pt = pt8p.tile([P, 2, 512], f8, tag="pt8", name="pt8_t")
                            if diag < 0:          # full pair
                                for j in range(2):
                                    nc.tensor.matmul(
                                        ps[:, j],
                                        kt_t[:, (ks0 + j) * P:
                                             (ks0 + j + 1) * P],
                                        qt_t[:, jb * 512:(jb + 1) * 512],
                                        start=True, stop=True)
                                nc.scalar.activation(
                                    pt, ps, EXP, scale=EXP_SCALE)
                            else:
                                m0 = diag      # 0 or 2
                                q00, q01 = m0 * P, (m0 + 1) * P
                                for j in range(2):
                                    nc.tensor.matmul(
                                        ps[:, j, q00:],
                                        kt_t[:, (ks0 + j) * P:
                                             (ks0 + j + 1) * P],
                                        qt_t[:, jb * 512 + q00:
                                             (jb + 1) * 512],
                                        start=True, stop=True)
                                nc.scalar.activation(
                                    pt[:, :, q00:], ps[:, :, q00:],
                                    EXP, scale=EXP_SCALE)
                                nc.gpsimd.memset(pt[:, 1, q00:q01], 0.0)
                                with nc.allow_low_precision("mask"):
                                    nc.gpsimd.tensor_tensor(
                                        pt[:, 0, q00:q01],
                                        pt[:, 0, q00:q01],
                                        tri8_t, MULT)
                                    nc.gpsimd.tensor_tensor(
                                        pt[:, 1, q01:q01 + P],
                                        pt[:, 1, q01:q01 + P],
                                        tri8_t, MULT)
                            st["pts"][kp] = (pt, max(0, diag) * P)

                        def av8_step(kp, npair):
                            pt, q0 = st["pts"][kp]
                            nc.tensor.matmul(
                                st["po"][:, q0:],
                                v8_t[:, 2 * kp:2 * kp + 2, h, :],
                                pt[:, :, q0:],
                                start=(kp == 0), stop=(kp == npair - 1),
                                perf_mode=DR)

                        def rs8_burst(npair):
                            # all rowsums back-to-back: constant ones8
                            # stationary is reloaded only once
                            for kp in range(npair):
                                pt, q0 = st["pts"].pop(kp)
                                nc.tensor.matmul(
                                    st["rs"][:, q0:], ones8_t, pt[:, :, q0:],
                                    start=(kp == 0), stop=(kp == npair - 1),
                                    perf_mode=DR)

                        def norm(jb):
                            po_, rs_ = st["po"], st["rs"]
                            recip = rcp.tile([P, 512], f32, tag="rc", name="recip_t")
                            with nc.allow_low_precision("recip"):
                                nc.vector.reciprocal_approx_fast(recip, rs_)
                            with nc.allow_low_precision("fp8 quant"):
                                if jb == 0:
                                    nc.vector.tensor_tensor(
                                        ot16_t[:, h], po_[:, 0:P],
                                        recip[:, 0:P], MULT)
                                    nc.vector.tensor_tensor(
                                        ot8_t[:, h, P:512], po_[:, P:],
                                        recip[:, P:], MULT)
                                else:
                                    nc.vector.tensor_tensor(
                                        ot8_t[:, h, jb * 512:(jb + 1) * 512],
                                        po_, recip, MULT)

                        # jb0 bf16 path, software-pipelined
                        steps.append(lambda: (new_block(), s16_step(0),
                                              s16_step(1)))
                        steps.append(lambda: (s16_step(2), av16_step(0)))
                        steps.append(lambda: (s16_step(3), av16_step(1)))
                        steps.append(lambda: (av16_step(2), av16_step(3)))
                        steps.append(lambda: norm(0))
                        # fp8 jbs
                        for jb in range(1, NTB):
                            npair = 2 * (jb + 1)
                            steps.append(lambda jb=jb: (new_block(),
                                                        s8_step(0, jb)))
                            for kp in range(npair):
                                def step(kp=kp, jb=jb, npair=npair):
                                    if kp + 1 < npair:
                                        s8_step(kp + 1, jb)
                                    av8_step(kp, npair)
                                steps.append(step)
                            steps.append(lambda npair=npair: rs8_burst(npair))
                            steps.append(lambda jb=jb: norm(jb))
                        return steps

                    with tc.tile_pool(name="psS", bufs=2,
                                      space="PSUM") as psSp:
                        tiles = load_head(0)
                        for s in proj_steps(tiles):
                            s()
                        for h in range(HG):
                            asteps = attn_steps(h, tiles)
                            if h + 1 < HG:
                                tiles = load_head(h + 1)
                                psteps = proj_steps(tiles)
                            else:
                                psteps = []
                            na, pi = len(asteps), 0
                            for i, s in enumerate(asteps):
                                s()
                                want = (i + 1) * len(psteps) // na
                                while pi < want:
                                    psteps[pi]()
                                    pi += 1

                    # ---------------- output projection ----------------
                    with tc.tile_pool(name="wo16p", bufs=4) as wo16p, \
                         tc.tile_pool(name="cstage", bufs=4) as cstage, \
                         tc.tile_pool(name="psC", bufs=4, space="PSUM") as psC:
                        # ob pairs share each stationary (halves LDW reloads)
                        for obp in range(2):
                            wo16_a = wo16p.tile([P, HG, 512], bf16,
                                                tag="wo16", name="wo16_a")
                            nc.sync.dma_start(
                                wo16_a,
                                wo16_d[:, :, (2 * obp) * 512:
                                       (2 * obp + 1) * 512])
                            wo16_b = wo16p.tile([P, HG, 512], bf16,
                                                tag="wo16", name="wo16_b")
                            nc.sync.dma_start(
                                wo16_b,
                                wo16_d[:, :, (2 * obp + 1) * 512:
                                       (2 * obp + 2) * 512])
                            for tch in range(NTC):
                                ps2 = [psC.tile([P, 512], f32, tag="psc",
                                                name=f"psc{j}")
                                       for j in range(2)]
                                if tch == 0:
                                    for hh in range(HG):
                                        for j, wot in enumerate(
                                                (wo16_a, wo16_b)):
                                            nc.tensor.matmul(
                                                ps2[j], ot16_t[:, hh],
                                                wot[:, hh],
                                                start=(hh == 0),
                                                stop=(hh == HG - 1))
                                else:
                                    for g2 in range(HG // 2):
                                        for j in range(2):
                                            ob = 2 * obp + j
                                            nc.tensor.matmul(
                                                ps2[j],
                                                ot8_t[:, 2 * g2:2 * g2 + 2,
                                                      tch * P:(tch + 1) * P],
                                                wo_t[:, 2 * g2:2 * g2 + 2,
                                                     ob * 512:(ob + 1) * 512],
                                                start=(g2 == 0),
                                                stop=(g2 == HG // 2 - 1),
                                                perf_mode=DR)
                                for j in range(2):
                                    ob = 2 * obp + j
                                    st = cstage.tile([P, 512], f32, tag="cst",
                                                     name="cst_t")
                                    if j == 0:
                                        nc.scalar.activation(st, ps2[j], COPY,
                                                             scale=OSC)
                                    else:
                                        with nc.allow_low_precision("osc"):
                                            nc.vector.tensor_scalar_mul(
                                                st, ps2[j], OSC)
                                    nc.sync.dma_start(
                                        out_d[tch * P:(tch + 1) * P,
                                              ob * 512:(ob + 1) * 512], st)

    nc.finalize()
    return nc


VMODE = "fp8"


def _get_nc():
    global _CACHED
    if _CACHED is None:
        _CACHED = _build()
    return _CACHED


def _prep_inputs(x, W_qkv, W_out, vmode=None):
    """Host-side shard + quantize + layout prep. Returns per-core input maps."""
    import ml_dtypes
    E4 = ml_dtypes.float8_e4m3
    BF = ml_dtypes.bfloat16
    f32 = np.float32
    x = np.asarray(x, dtype=f32)
    W_qkv = np.asarray(W_qkv, dtype=f32)
    W_out = np.asarray(W_out, dtype=f32)

    k_idx = np.arange(P)
    tri = (np.arange(P)[None, :] >= k_idx[:, None]).astype(f32)  # [k, q]
    tri8 = tri.astype(E4)
    tri16 = tri.astype(BF)
    ones8 = np.full((P, 2, P), 1.0 / 16.0, dtype=E4)
    ones16 = np.full((P, P), 1.0 / 16.0, dtype=BF)

    per_g = {}
    for g in range(2):
        sl = slice(g * HG * D, (g + 1) * HG * D)
        wq = (W_qkv[0 * C:1 * C][sl] * 512.0)
        wk = (W_qkv[1 * C:2 * C][sl] * 512.0)
        wv = (W_qkv[2 * C:3 * C][sl] * 512.0)
        # [h, p, cs, m]: element = w[h*128+m, cs*128+p]
        wq_a = np.ascontiguousarray(
            wq.reshape(HG, D, NCS, P).transpose(0, 3, 2, 1)).astype(E4)
        wk_a = np.ascontiguousarray(
            wk.reshape(HG, D, NCS, P).transpose(0, 3, 2, 1)).astype(E4)
        # [p, cs, hm]: element = wv[hm, cs*128+p]
        wv_r = np.ascontiguousarray(
            wv.reshape(HG * D, NCS, P).transpose(2, 1, 0))
        wv_a = wv_r.astype(E4)
        wv16_a = wv_r.astype(BF)
        # [p(d), h, o]: element = W_out[o, g*1024 + h*128 + d] * 512
        wo_r = np.ascontiguousarray(
            (W_out[:, sl] * 512.0).reshape(C, HG, D).transpose(2, 1, 0))
        wo_a = wo_r.astype(E4)
        wo16_a = wo_r.astype(BF)
        per_g[g] = (wq_a, wk_a, wv_a, wv16_a, wo_a, wo16_a)

    in_maps = []
    for core in range(8):
        b, g = divmod(core, 2)
        xT = np.ascontiguousarray(x[b].T)                      # [C, T]
        xt8 = xT.reshape(NPR, 2, P, T).transpose(0, 2, 1, 3)   # [pr, p, 2, T]
        xt8 = np.ascontiguousarray(xt8).astype(E4)
        xt16 = np.ascontiguousarray(
            xT[:, :P].reshape(NCS, P, P).transpose(1, 0, 2)).astype(BF)
        wq_a, wk_a, wv_a, wv16_a, wo_a, wo16_a = per_g[g]
        im = {
            "xt": xt8, "xt16": xt16, "wq": wq_a, "wk": wk_a,
            "wv": wv_a, "wv16": wv16_a, "wo": wo_a, "wo16": wo16_a,
            "tri8": tri8, "tri16": tri16, "ones8": ones8, "ones16": ones16,
        }
        in_maps.append(im)
    return in_maps


def kernel(x, W_qkv, W_out, *, trace=False, trace_cores=None):
    from concourse.bass_utils import run_bass_kernel_spmd

    nc = _get_nc()
    in_maps = _prep_inputs(x, W_qkv, W_out)
    r = run_bass_kernel_spmd(
        nc, in_maps, core_ids=list(range(8)),
        trace=trace, trace_cores=trace_cores)

    out = np.empty((B, T, C), dtype=np.float32)
    for b in range(B):
        out[b] = r.results[2 * b]["out"] + r.results[2 * b + 1]["out"]
    if trace:
        kernel.last_results = r
    return out


# revision 15
# speedup vs baseline: 6.3477x; 6.3477x over previous
"""Causal self-attention Trainium2 kernel (8 NeuronCores, SPMD), fp8 DoubleRow.

Problem (hardcoded): x [4, 2048, 2048] f32, W_qkv [6144, 2048], W_out [2048, 2048],
16 heads x 128 dim, causal softmax attention + output projection.

Sharding: core c = 2*b + g handles batch b (4) and head-group g (2 groups of 8
heads).  Host sums the two partial out-projections per batch element.

Precision scheme (validated vs reference, rel err ~1.0e-2 on CPU emulation):
- All projections run as fp8e4 (e4m3) DoubleRow matmuls (0.5 cycles/row):
  x quantized e4m3, weights quantized e4m3 after x512 scaling.
- Attention S/AV/rowsum in fp8 (AV+rowsum DoubleRow over k-chunk pairs).
- Early rows are error-amplified (few-key softmax), so the first q-block
  (t<512) runs a bf16 attention path: S from bf16 Q/K, bf16 P, bf16 V
  (V for k<128 additionally recomputed via a bf16 x@Wv projection), and the
  first 128 output rows go through a bf16 output projection.

Scales: qt8/kt8/v8 = psum * 4/512 (=4Q etc.), exp scale = (1/sqrt(128))/16,
rowsum ones = 1/16 so po*recip = 64*O, out-proj psum = 64*O*512*Wo = 32768*out.

No DRAM intermediates: Q/K/V/O live in SBUF across the fused pipeline.
"""

import math

import numpy as np

B = 4
T = 2048
C = 2048
H = 16          # total heads
HG = 8          # heads per core (tensor-parallel group)
D = 128         # head dim
P = 128         # partitions
NCS = C // P    # 16 contraction subtiles
NPR = NCS // 2  # 8 contraction pair-tiles
NTC = T // P    # 16 T chunks of 128
NTB = T // 512  # 4 T blocks of 512
SCALE = 1.0 / math.sqrt(D)
EXP_SCALE = SCALE / 16.0
QSC = 4.0 / 512.0       # psum -> qt8/kt8/v8 scale
OSC = 2.0 ** -15        # out-proj psum -> f32 out

_CACHED = None


def _build(phases="abc", repeat=1, vmode=None):
    import concourse.mybir as mybir
    from concourse import bacc
    from concourse.tile import TileContext

    f32 = mybir.dt.float32
    f32r = mybir.dt.float32r
    f8 = mybir.dt.float8e4
    bf16 = mybir.dt.bfloat16
    EXP = mybir.ActivationFunctionType.Exp
    COPY = mybir.ActivationFunctionType.Copy
    MULT = mybir.AluOpType.mult
    DR = mybir.MatmulPerfMode.DoubleRow

    nc = bacc.Bacc("TRN2", target_bir_lowering=False)

    xt_d = nc.dram_tensor("xt", [NPR, P, 2, T], f8, kind="ExternalInput")
    xt16_d = nc.dram_tensor("xt16", [P, NCS, P], bf16, kind="ExternalInput")
    wq_d = nc.dram_tensor("wq", [HG, P, NCS, D], f8, kind="ExternalInput")
    wk_d = nc.dram_tensor("wk", [HG, P, NCS, D], f8, kind="ExternalInput")
    wv_d = nc.dram_tensor("wv", [P, NCS, HG * D], f8, kind="ExternalInput")
    wv16_d = nc.dram_tensor("wv16", [P, NCS, HG * D], bf16, kind="ExternalInput")
    wo_d = nc.dram_tensor("wo", [P, HG, C], f8, kind="ExternalInput")
    wo16_d = nc.dram_tensor("wo16", [P, HG, C], bf16, kind="ExternalInput")
    tri8_d = nc.dram_tensor("tri8", [P, P], f8, kind="ExternalInput")
    tri16_d = nc.dram_tensor("tri16", [P, P], bf16, kind="ExternalInput")
    ones8_d = nc.dram_tensor("ones8", [P, 2, P], f8, kind="ExternalInput")
    ones16_d = nc.dram_tensor("ones16", [P, P], bf16, kind="ExternalInput")
    out_d = nc.dram_tensor("out", [T, C], f32, kind="ExternalOutput")

    with TileContext(nc) as tc:
        with tc.tile_pool(name="persist", bufs=1) as persist:
            # constants
            tri8_t = persist.tile([P, P], f8, tag="tri8")
            nc.sync.dma_start(tri8_t, tri8_d[:])
            tri16_t = persist.tile([P, P], bf16, tag="tri16")
            nc.sync.dma_start(tri16_t, tri16_d[:])
            ones8_t = persist.tile([P, 2, P], f8, tag="ones8")
            nc.sync.dma_start(ones8_t, ones8_d[:])
            ones16_t = persist.tile([P, P], bf16, tag="ones16")
            nc.sync.dma_start(ones16_t, ones16_d[:])
            # x^T fp8 pair tiles + bf16 first 128 cols
            xt16_t = persist.tile([P, NCS, P], bf16, tag="xt16")
            nc.sync.dma_start(xt16_t, xt16_d[:])
            xt = []
            for pr in range(NPR):
                t_ = persist.tile([P, 2, T], f8, tag=f"xt{pr}")
                nc.sync.dma_start(t_, xt_d[pr])
                xt.append(t_)
            # resident weights
            wv_t = persist.tile([P, NCS, HG * D], f8, tag="wv")
            nc.sync.dma_start(wv_t, wv_d[:])
            wo_t = persist.tile([P, HG, C], f8, tag="wo")
            nc.sync.dma_start(wo_t, wo_d[:])
            # V for all heads (AV lhsT), fp8 + bf16 early chunks
            v8_t = persist.tile([P, NTC, HG, D], f8, tag="v8")
            v16_t = persist.tile([P, 4, HG, D], bf16, tag="v16")
            # output activations
            ot8_t = persist.tile([P, HG, T], f8, tag="ot8")
            ot16_t = persist.tile([P, HG, P], bf16, tag="ot16")

            for _rep in range(repeat):
                with tc.tile_pool(name="psA", bufs=2, space="PSUM") as psA, \
                     tc.tile_pool(name="psO", bufs=2, space="PSUM") as psO, \
                     tc.tile_pool(name="hw", bufs=2) as hwp, \
                     tc.tile_pool(name="hqk", bufs=2) as hqk, \
                     tc.tile_pool(name="hq16", bufs=2) as hq16, \
                     tc.tile_pool(name="pt8", bufs=10) as pt8p, \
                     tc.tile_pool(name="pt16", bufs=3) as pt16p, \
                     tc.tile_pool(name="cstage", bufs=4) as cstage, \
                     tc.tile_pool(name="rc", bufs=4) as rcp:

                    # ---------------- V projection (all heads) ----------------
                    with tc.tile_pool(name="wv16p", bufs=1) as wv16p, \
                         tc.tile_pool(name="psV", bufs=2, space="PSUM") as psV:
                        wv16_t = wv16p.tile([P, NCS, HG * D], bf16, tag="wv16")
                        nc.sync.dma_start(wv16_t, wv16_d[:])
                        for tch in range(NTC):
                            ps = psV.tile([P, 2, 512], f32, tag="psv")
                            for half in range(2):
                                if tch == 0:
                                    # bf16 V proj for k<128 (accuracy)
                                    for cs in range(NCS):
                                        nc.tensor.matmul(
                                            ps[:, half],
                                            xt16_t[:, cs],
                                            wv16_t[:, cs,
                                                   half * 512:(half + 1) * 512],
                                            start=(cs == 0), stop=(cs == NCS - 1))
                                else:
                                    for pr in range(NPR):
                                        nc.tensor.matmul(
                                            ps[:, half],
                                            xt[pr][:, :, tch * P:(tch + 1) * P],
                                            wv_t[:, 2 * pr:2 * pr + 2,
                                                 half * 512:(half + 1) * 512],
                                            start=(pr == 0), stop=(pr == NPR - 1),
                                            perf_mode=DR)
                            with nc.allow_low_precision("fp8 quant"):
                                nc.vector.tensor_scalar_mul(
                                    v8_t[:, tch], ps, QSC)
                                if tch < 4:
                                    nc.vector.tensor_scalar_mul(
                                        v16_t[:, tch], ps, QSC)

                    def c_fp8_step(tch, obp):
                        # fp8 out-proj for one (row-chunk, ob-pair); resident
                        # wo only, psums borrowed from the (idle) proj pool
                        ps2 = [psA.tile([P, 512], f32, tag="psa",
                                        name=f"psc8_{j}") for j in range(2)]
                        for g2 in range(HG // 2):
                            for j in range(2):
                                ob = 2 * obp + j
                                nc.tensor.matmul(
                                    ps2[j],
                                    ot8_t[:, 2 * g2:2 * g2 + 2,
                                          tch * P:(tch + 1) * P],
                                    wo_t[:, 2 * g2:2 * g2 + 2,
                                         ob * 512:(ob + 1) * 512],
                                    start=(g2 == 0),
                                    stop=(g2 == HG // 2 - 1),
                                    perf_mode=DR)
                        for j in range(2):
                            ob = 2 * obp + j
                            st_ = cstage.tile([P, 512], f32, tag="cst",
                                              name="cst8_t")
                            if j == 0:
                                nc.scalar.activation(st_, ps2[j], COPY,
                                                     scale=OSC)
                            else:
                                with nc.allow_low_precision("osc"):
                                    nc.vector.tensor_scalar_mul(
                                        st_, ps2[j], OSC)
                            nc.sync.dma_start(
                                out_d[tch * P:(tch + 1) * P,
                                      ob * 512:(ob + 1) * 512], st_)

                    # ------- head pipeline: attn(h) interleaved with proj(h+1)
                    def load_head(h):
                        wq_t = hwp.tile([P, NCS, D], f8, tag="wq", name="wq_t")
                        nc.sync.dma_start(wq_t, wq_d[h])
                        wk_t = hwp.tile([P, NCS, D], f8, tag="wk", name="wk_t")
                        nc.sync.dma_start(wk_t, wk_d[h])
                        qt_t = hqk.tile([P, T], f8, tag="qt", name="qt_t")
                        kt_t = hqk.tile([P, T], f8, tag="kt", name="kt_t")
                        q16_t = hq16.tile([P, 512], bf16, tag="q16", name="q16_t")
                        k16_t = hq16.tile([P, 512], bf16, tag="k16", name="k16_t")
                        return (wq_t, wk_t, qt_t, kt_t, q16_t, k16_t)

                    def proj_steps(tiles):
                        wq_t, wk_t, qt_t, kt_t, q16_t, k16_t = tiles
                        steps = []
                        for w_t, dst8, dst16 in ((wq_t, qt_t, q16_t),
                                                 (wk_t, kt_t, k16_t)):
                            for tbp in range(2):
                                def step(w_t=w_t, dst8=dst8, dst16=dst16,
                                         tbp=tbp):
                                    tb0, tb1 = 2 * tbp, 2 * tbp + 1
                                    psa = psA.tile([P, 512], f32, tag="psa",
                                                   name="ps_a0")
                                    psb = psA.tile([P, 512], f32, tag="psa",
                                                   name="ps_a1")
                                    # both tb blocks share each stationary
                                    for pr in range(NPR):
                                        for ps_, tb in ((psa, tb0),
                                                        (psb, tb1)):
                                            nc.tensor.matmul(
                                                ps_,
                                                w_t[:, 2 * pr:2 * pr + 2],
                                                xt[pr][:, :, tb * 512:
                                                       (tb + 1) * 512],
                                                start=(pr == 0),
                                                stop=(pr == NPR - 1),
                                                perf_mode=DR)
                                    with nc.allow_low_precision("fp8 quant"):
                                        for ps_, tb in ((psa, tb0),
                                                        (psb, tb1)):
                                            nc.vector.tensor_scalar_mul(
                                                dst8[:, tb * 512:
                                                     (tb + 1) * 512],
                                                ps_, QSC)
                                        if tbp == 0:
                                            nc.vector.tensor_scalar_mul(
                                                dst16, psa, QSC)
                                steps.append(step)
                        return steps

                    def attn_steps(h, tiles):
                        wq_t, wk_t, qt_t, kt_t, q16_t, k16_t = tiles
                        st = {}
                        steps = []

                        def new_block():
                            st["po"] = psO.tile([P, 512], f32, tag="po", name="po_t")
                            st["rs"] = psO.tile([P, 512], f32, tag="po", name="rs_t")
                            st["pts"] = {}

                        def s16_step(m):
                            q0 = m * P
                            if m % 2 == 0:
                                st["ps16"] = psSp.tile([P, 2, 512], f32, tag="pss", name="ps16_t")
                            ps = st["ps16"]
                            nc.tensor.matmul(
                                ps[:, m % 2, q0:], k16_t[:, m * P:(m + 1) * P],
                                q16_t[:, q0:], start=True, stop=True)
                            if m % 2 == 1:
                                pt = pt16p.tile([P, 2, 512], bf16, tag="pt16", name="pt16_t")
                                for j in (m - 1, m):
                                    jq0 = j * P
                                    nc.scalar.activation(
                                        pt[:, j % 2, jq0:], ps[:, j % 2, jq0:],
                                        EXP, scale=EXP_SCALE)
                                    with nc.allow_low_precision("mask"):
                                        nc.gpsimd.tensor_tensor(
                                            pt[:, j % 2, jq0:jq0 + P],
                                            pt[:, j % 2, jq0:jq0 + P],
                                            tri16_t, MULT)
                                    st["pts"][j] = pt

                        def av16_step(m):
                            q0 = m * P
                            pt = st["pts"].pop(m)
                            nc.tensor.matmul(
                                st["po"][:, q0:], v16_t[:, m, h],
                                pt[:, m % 2, q0:],
                                start=(m == 0), stop=(m == 3))
                            nc.tensor.matmul(
                                st["rs"][:, q0:], ones16_t, pt[:, m % 2, q0:],
                                start=(m == 0), stop=(m == 3))

                        def s8_step(kp, jb):
                            ks0 = 2 * kp
                            diag = ks0 - 4 * jb
                            ps = psSp.tile([P, 2, 512], f32, tag="pss", name="ps8_t")
                            pt = pt8p.tile([P, 2, 512], f8, tag="pt8", name="pt8_t")
                            if diag < 0:          # full pair
                                for j in range(2):
                                    nc.tensor.matmul(
                                        ps[:, j],
                                        kt_t[:, (ks0 + j) * P:
                                             (ks0 + j + 1) * P],
                                        qt_t[:, jb * 512:(jb + 1) * 512],
                                        start=True, stop=True)
                                nc.scalar.activation(
                                    pt, ps, EXP, scale=EXP_SCALE)
                            else:
                                m0 = diag      # 0 or 2
                                q00, q01 = m0 * P, (m0 + 1) * P
                                for j in range(2):
                                    nc.tensor.matmul(
                                        ps[:, j, q00:],
                                        kt_t[:, (ks0 + j) * P:
                                             (ks0 + j + 1) * P],
                                        qt_t[:, jb * 512 + q00:
                                             (jb + 1) * 512],
                                        start=True, stop=True)
                                nc.scalar.activation(
                                    pt[:, :, q00:], ps[:, :, q00:],
                                    EXP, scale=EXP_SCALE)
                                nc.gpsimd.memset(pt[:, 1, q00:q01], 0.0)
                                with nc.allow_low_precision("mask"):
                                    nc.gpsimd.tensor_tensor(
                                        pt[:, 0, q00:q01],
                                        pt[:, 0, q00:q01],
                                        tri8_t, MULT)
                                    nc.gpsimd.tensor_tensor(
                                        pt[:, 1, q01:q01 + P],
                                        pt[:, 1, q01:q01 + P],
                                        tri8_t, MULT)
                            st["pts"][kp] = (pt, max(0, diag) * P)

                        def av8_step(kp, npair):
                            pt, q0 = st["pts"][kp]
                            nc.tensor.matmul(
                                st["po"][:, q0:],
                                v8_t[:, 2 * kp:2 * kp + 2, h, :],
                                pt[:, :, q0:],
                                start=(kp == 0), stop=(kp == npair - 1),
                                perf_mode=DR)

                        def rs8_burst(npair):
                            # all rowsums back-to-back: constant ones8
                            # stationary is reloaded only once
                            for kp in range(npair):
                                pt, q0 = st["pts"].pop(kp)
                                nc.tensor.matmul(
                                    st["rs"][:, q0:], ones8_t, pt[:, :, q0:],
                                    start=(kp == 0), stop=(kp == npair - 1),
                                    perf_mode=DR)

                        def norm(jb):
                            po_, rs_ = st["po"], st["rs"]
                            recip = rcp.tile([P, 512], f32, tag="rc", name="recip_t")
                            with nc.allow_low_precision("recip"):
                                nc.vector.reciprocal_approx_fast(recip, rs_)
                            with nc.allow_low_precision("fp8 quant"):
                                if jb == 0:
                                    nc.vector.tensor_tensor(
                                        ot16_t[:, h], po_[:, 0:P],
                                        recip[:, 0:P], MULT)
                                    nc.vector.tensor_tensor(
                                        ot8_t[:, h, P:512], po_[:, P:],
                                        recip[:, P:], MULT)
                                else:
                                    nc.vector.tensor_tensor(
                                        ot8_t[:, h, jb * 512:(jb + 1) * 512],
                                        po_, recip, MULT)

                        # jb0 bf16 path, software-pipelined
                        steps.append((lambda: (new_block(), s16_step(0),
                                               s16_step(1)), None))
                        steps.append((lambda: (s16_step(2), av16_step(0)),
                                      None))
                        steps.append((lambda: (s16_step(3), av16_step(1)),
                                      None))
                        steps.append((lambda: (av16_step(2), av16_step(3)),
                                      None))
                        steps.append((lambda: norm(0), 0))
                        # fp8 jbs
                        for jb in range(1, NTB):
                            npair = 2 * (jb + 1)
                            steps.append((lambda jb=jb: (new_block(),
                                                         s8_step(0, jb)),
                                          None))
                            for kp in range(npair):
                                def step(kp=kp, jb=jb, npair=npair):
                                    if kp + 1 < npair:
                                        s8_step(kp + 1, jb)
                                    av8_step(kp, npair)
                                steps.append((step, None))
                            steps.append((lambda npair=npair:
                                          rs8_burst(npair), None))
                            steps.append((lambda jb=jb: norm(jb), jb))
                        return steps

                    with tc.tile_pool(name="psS", bufs=2,
                                      space="PSUM") as psSp:
                        tiles = load_head(0)
                        for s in proj_steps(tiles):
                            s()
                        for h in range(HG):
                            asteps = attn_steps(h, tiles)
                            if h + 1 < HG:
                                tiles = load_head(h + 1)
                                psteps = proj_steps(tiles)
                            else:
                                psteps = []
                            na, pi = len(asteps), 0
                            for i, (s, mark) in enumerate(asteps):
                                s()
                                if h == HG - 1 and mark is not None \
                                        and mark < NTB - 1:
                                    # rows of block `mark` are final: run
                                    # their fp8 out-proj now (tch0 is bf16,
                                    # handled in the tail)
                                    t0 = 4 * mark if mark else 1
                                    for tch in range(t0, 4 * mark + 4):
                                        for obp in range(2):
                                            c_fp8_step(tch, obp)
                                want = (i + 1) * len(psteps) // na
                                while pi < want:
                                    psteps[pi]()
                                    pi += 1

                    # ---------------- output projection ----------------
                    with tc.tile_pool(name="wo16p", bufs=4) as wo16p, \
                         tc.tile_pool(name="psC", bufs=4, space="PSUM") as psC:
                        # ob pairs share each stationary (halves LDW reloads)
                        for obp in range(2):
                            wo16_a = wo16p.tile([P, HG, 512], bf16,
                                                tag="wo16", name="wo16_a")
                            nc.sync.dma_start(
                                wo16_a,
                                wo16_d[:, :, (2 * obp) * 512:
                                       (2 * obp + 1) * 512])
                            wo16_b = wo16p.tile([P, HG, 512], bf16,
                                                tag="wo16", name="wo16_b")
                            nc.sync.dma_start(
                                wo16_b,
                                wo16_d[:, :, (2 * obp + 1) * 512:
                                       (2 * obp + 2) * 512])
                            for tch in [0] + list(range(12, NTC)):
                                ps2 = [psC.tile([P, 512], f32, tag="psc",
                                                name=f"psc{j}")
                                       for j in range(2)]
                                if tch == 0:
                                    for hh in range(HG):
                                        for j, wot in enumerate(
                                                (wo16_a, wo16_b)):
                                            nc.tensor.matmul(
                                                ps2[j], ot16_t[:, hh],
                                                wot[:, hh],
                                                start=(hh == 0),
                                                stop=(hh == HG - 1))
                                else:
                                    for g2 in range(HG // 2):
                                        for j in range(2):
                                            ob = 2 * obp + j
                                            nc.tensor.matmul(
                                                ps2[j],
                                                ot8_t[:, 2 * g2:2 * g2 + 2,
                                                      tch * P:(tch + 1) * P],
                                                wo_t[:, 2 * g2:2 * g2 + 2,
                                                     ob * 512:(ob + 1) * 512],
                                                start=(g2 == 0),
                                                stop=(g2 == HG // 2 - 1),
                                                perf_mode=DR)
                                for j in range(2):
                                    ob = 2 * obp + j
                                    st = cstage.tile([P, 512], f32, tag="cst",
                                                     name="cst_t")
                                    if j == 0:
                                        nc.scalar.activation(st, ps2[j], COPY,
                                                             scale=OSC)
                                    else:
                                        with nc.allow_low_precision("osc"):
                                            nc.vector.tensor_scalar_mul(
                                                st, ps2[j], OSC)
                                    nc.sync.dma_start(
                                        out_d[tch * P:(tch + 1) * P,
                                              ob * 512:(ob + 1) * 512], st)

    nc.finalize()
    return nc


VMODE = "fp8"


def _get_nc():
    global _CACHED
    if _CACHED is None:
        _CACHED = _build()
    return _CACHED


def _prep_inputs(x, W_qkv, W_out, vmode=None):
    """Host-side shard + quantize + layout prep. Returns per-core input maps."""
    import ml_dtypes
    E4 = ml_dtypes.float8_e4m3
    BF = ml_dtypes.bfloat16
    f32 = np.float32
    x = np.asarray(x, dtype=f32)
    W_qkv = np.asarray(W_qkv, dtype=f32)
    W_out = np.asarray(W_out, dtype=f32)

    k_idx = np.arange(P)
    tri = (np.arange(P)[None, :] >= k_idx[:, None]).astype(f32)  # [k, q]
    tri8 = tri.astype(E4)
    tri16 = tri.astype(BF)
    ones8 = np.full((P, 2, P), 1.0 / 16.0, dtype=E4)
    ones16 = np.full((P, P), 1.0 / 16.0, dtype=BF)

    per_g = {}
    for g in range(2):
        sl = slice(g * HG * D, (g + 1) * HG * D)
        wq = (W_qkv[0 * C:1 * C][sl] * 512.0)
        wk = (W_qkv[1 * C:2 * C][sl] * 512.0)
        wv = (W_qkv[2 * C:3 * C][sl] * 512.0)
        # [h, p, cs, m]: element = w[h*128+m, cs*128+p]
        wq_a = np.ascontiguousarray(
            wq.reshape(HG, D, NCS, P).transpose(0, 3, 2, 1)).astype(E4)
        wk_a = np.ascontiguousarray(
            wk.reshape(HG, D, NCS, P).transpose(0, 3, 2, 1)).astype(E4)
        # [p, cs, hm]: element = wv[hm, cs*128+p]
        wv_r = np.ascontiguousarray(
            wv.reshape(HG * D, NCS, P).transpose(2, 1, 0))
        wv_a = wv_r.astype(E4)
        wv16_a = wv_r.astype(BF)
        # [p(d), h, o]: element = W_out[o, g*1024 + h*128 + d] * 512
        wo_r = np.ascontiguousarray(
            (W_out[:, sl] * 512.0).reshape(C, HG, D).transpose(2, 1, 0))
        wo_a = wo_r.astype(E4)
        wo16_a = wo_r.astype(BF)
        per_g[g] = (wq_a, wk_a, wv_a, wv16_a, wo_a, wo16_a)

    in_maps = []
    for core in range(8):
        b, g = divmod(core, 2)
        xT = np.ascontiguousarray(x[b].T)                      # [C, T]
        xt8 = xT.reshape(NPR, 2, P, T).transpose(0, 2, 1, 3)   # [pr, p, 2, T]
        xt8 = np.ascontiguousarray(xt8).astype(E4)
        xt16 = np.ascontiguousarray(
            xT[:, :P].reshape(NCS, P, P).transpose(1, 0, 2)).astype(BF)
        wq_a, wk_a, wv_a, wv16_a, wo_a, wo16_a = per_g[g]
        im = {
            "xt": xt8, "xt16": xt16, "wq": wq_a, "wk": wk_a,
            "wv": wv_a, "wv16": wv16_a, "wo": wo_a, "wo16": wo16_a,
            "tri8": tri8, "tri16": tri16, "ones8": ones8, "ones16": ones16,
        }
        in_maps.append(im)
    return in_maps


def kernel(x, W_qkv, W_out, *, trace=False, trace_cores=None):
    from concourse.bass_utils import run_bass_kernel_spmd

    nc = _get_nc()
    in_maps = _prep_inputs(x, W_qkv, W_out)
    r = run_bass_kernel_spmd(
        nc, in_maps, core_ids=list(range(8)),
        trace=trace, trace_cores=trace_cores)

    out = np.empty((B, T, C), dtype=np.float32)
    for b in range(B):
        out[b] = r.results[2 * b]["out"] + r.results[2 * b + 1]["out"]
    if trace:
        kernel.last_results = r
    return out


# revision 18
# speedup vs baseline: 14.5754x; 2.2962x over previous
"""Causal self-attention Trainium2 kernel (8 NeuronCores, SPMD), fp8 DoubleRow.

Problem (hardcoded): x [4, 2048, 2048] f32, W_qkv [6144, 2048], W_out [2048, 2048],
16 heads x 128 dim, causal softmax attention + output projection.

Sharding: core c = 2*b + g handles batch b (4) and head-group g (2 groups of 8
heads).  Host sums the two partial out-projections per batch element.

Precision scheme (validated vs reference, rel err ~1.0e-2 on CPU emulation):
- All projections run as fp8e4 (e4m3) DoubleRow matmuls (0.5 cycles/row):
  x quantized e4m3, weights quantized e4m3 after x512 scaling.
- Attention S/AV/rowsum in fp8 (AV+rowsum DoubleRow over k-chunk pairs).
- Early rows are error-amplified (few-key softmax), so the first q-block
  (t<512) runs a bf16 attention path: S from bf16 Q/K, bf16 P, bf16 V
  (V for k<128 additionally recomputed via a bf16 x@Wv projection), and the
  first 128 output rows go through a bf16 output projection.

Scales: qt8/kt8/v8 = psum * 4/512 (=4Q etc.), exp scale = (1/sqrt(128))/16,
rowsum ones = 1/16 so po*recip = 64*O, out-proj psum = 64*O*512*Wo = 32768*out.

No DRAM intermediates: Q/K/V/O live in SBUF across the fused pipeline.
"""

import math

import numpy as np

B = 4
T = 2048
C = 2048
H = 16          # total heads
HG = 8          # heads per core (tensor-parallel group)
D = 128         # head dim
P = 128         # partitions
NCS = C // P    # 16 contraction subtiles
NPR = NCS // 2  # 8 contraction pair-tiles
NTC = T // P    # 16 T chunks of 128
NTB = T // 512  # 4 T blocks of 512
SCALE = 1.0 / math.sqrt(D)
EXP_SCALE = SCALE / 16.0
QSC = 4.0 / 512.0       # psum -> qt8/kt8/v8 scale
OSC = 2.0 ** -15        # out-proj psum -> f32 out

_CACHED = None


def _build(phases="abc", repeat=1, vmode=None):
    import concourse.mybir as mybir
    from concourse import bacc
    from concourse.tile import TileContext

    f32 = mybir.dt.float32
    f32r = mybir.dt.float32r
    f8 = mybir.dt.float8e4
    bf16 = mybir.dt.bfloat16
    EXP = mybir.ActivationFunctionType.Exp
    COPY = mybir.ActivationFunctionType.Copy
    MULT = mybir.AluOpType.mult
    DR = mybir.MatmulPerfMode.DoubleRow

    nc = bacc.Bacc("TRN2", target_bir_lowering=False)

    xt_d = nc.dram_tensor("xt", [NPR, P, 2, T], f8, kind="ExternalInput")
    xt16_d = nc.dram_tensor("xt16", [P, NCS, P], bf16, kind="ExternalInput")
    wq_d = nc.dram_tensor("wq", [HG, P, NCS, D], f8, kind="ExternalInput")
    wk_d = nc.dram_tensor("wk", [HG, P, NCS, D], f8, kind="ExternalInput")
    wv_d = nc.dram_tensor("wv", [P, NCS, HG * D], f8, kind="ExternalInput")
    wv16_d = nc.dram_tensor("wv16", [P, NCS, HG * D], bf16, kind="ExternalInput")
    wo_d = nc.dram_tensor("wo", [P, HG, C], f8, kind="ExternalInput")
    wo16_d = nc.dram_tensor("wo16", [P, HG, C], bf16, kind="ExternalInput")
    tri8_d = nc.dram_tensor("tri8", [P, P], f8, kind="ExternalInput")
    tri16_d = nc.dram_tensor("tri16", [P, P], bf16, kind="ExternalInput")
    ones8_d = nc.dram_tensor("ones8", [P, 2, P], f8, kind="ExternalInput")
    ones16_d = nc.dram_tensor("ones16", [P, P], bf16, kind="ExternalInput")
    out_d = nc.dram_tensor("out", [T, C], f32, kind="ExternalOutput")

    with TileContext(nc) as tc:
        with tc.tile_pool(name="persist", bufs=1) as persist:
            # x + wv first: they gate the fp8 V-projection that opens the
            # pipeline; constants are tiny; wo/wo16 are only needed late
            xt = []
            for pr in range(NPR):
                t_ = persist.tile([P, 2, T], f8, tag=f"xt{pr}")
                nc.sync.dma_start(t_, xt_d[pr])
                xt.append(t_)
            wv_t = persist.tile([P, NCS, HG * D], f8, tag="wv")
            nc.sync.dma_start(wv_t, wv_d[:])
            tri8_t = persist.tile([P, P], f8, tag="tri8")
            nc.sync.dma_start(tri8_t, tri8_d[:])
            tri16_t = persist.tile([P, P], bf16, tag="tri16")
            nc.sync.dma_start(tri16_t, tri16_d[:])
            ones8_t = persist.tile([P, 2, P], f8, tag="ones8")
            nc.sync.dma_start(ones8_t, ones8_d[:])
            ones16_t = persist.tile([P, P], bf16, tag="ones16")
            nc.sync.dma_start(ones16_t, ones16_d[:])
            xt16_t = persist.tile([P, NCS, P], bf16, tag="xt16")
            nc.sync.dma_start(xt16_t, xt16_d[:])
            wo_t = persist.tile([P, HG, C], f8, tag="wo")
            nc.sync.dma_start(wo_t, wo_d[:])
            # V for all heads (AV lhsT), fp8 + bf16 early chunks
            v8_t = persist.tile([P, NTC, HG, D], f8, tag="v8")
            v16_t = persist.tile([P, 4, HG, D], bf16, tag="v16")
            # output activations
            ot8_t = persist.tile([P, HG, T], f8, tag="ot8")
            ot16_t = persist.tile([P, HG, P], bf16, tag="ot16")

            for _rep in range(repeat):
                with tc.tile_pool(name="psA", bufs=2, space="PSUM") as psA, \
                     tc.tile_pool(name="psO", bufs=2, space="PSUM") as psO, \
                     tc.tile_pool(name="hw", bufs=2) as hwp, \
                     tc.tile_pool(name="hqk", bufs=2) as hqk, \
                     tc.tile_pool(name="hq16", bufs=2) as hq16, \
                     tc.tile_pool(name="pt8", bufs=10) as pt8p, \
                     tc.tile_pool(name="pt16", bufs=3) as pt16p, \
                     tc.tile_pool(name="cstage", bufs=4) as cstage, \
                     tc.tile_pool(name="rc", bufs=4) as rcp:

                    def c_fp8_step(tch, obp):
                        # fp8 out-proj for one (row-chunk, ob-pair); resident
                        # wo only, psums borrowed from the (idle) proj pool
                        ps2 = [psA.tile([P, 512], f32, tag="psa",
                                        name=f"psc8_{j}") for j in range(2)]
                        for g2 in range(HG // 2):
                            for j in range(2):
                                ob = 2 * obp + j
                                nc.tensor.matmul(
                                    ps2[j],
                                    ot8_t[:, 2 * g2:2 * g2 + 2,
                                          tch * P:(tch + 1) * P],
                                    wo_t[:, 2 * g2:2 * g2 + 2,
                                         ob * 512:(ob + 1) * 512],
                                    start=(g2 == 0),
                                    stop=(g2 == HG // 2 - 1),
                                    perf_mode=DR)
                        for j in range(2):
                            ob = 2 * obp + j
                            st_ = cstage.tile([P, 512], f32, tag="cst",
                                              name="cst8_t")
                            if j == 0:
                                nc.scalar.activation(st_, ps2[j], COPY,
                                                     scale=OSC)
                            else:
                                with nc.allow_low_precision("osc"):
                                    nc.vector.tensor_scalar_mul(
                                        st_, ps2[j], OSC)
                            nc.sync.dma_start(
                                out_d[tch * P:(tch + 1) * P,
                                      ob * 512:(ob + 1) * 512], st_)

                    # ------- head pipeline: attn(h) interleaved with proj(h+1)
                    def load_head(h):
                        wq_t = hwp.tile([P, NCS, D], f8, tag="wq", name="wq_t")
                        nc.sync.dma_start(wq_t, wq_d[h])
                        wk_t = hwp.tile([P, NCS, D], f8, tag="wk", name="wk_t")
                        nc.sync.dma_start(wk_t, wk_d[h])
                        qt_t = hqk.tile([P, T], f8, tag="qt", name="qt_t")
                        kt_t = hqk.tile([P, T], f8, tag="kt", name="kt_t")
                        q16_t = hq16.tile([P, 512], bf16, tag="q16", name="q16_t")
                        k16_t = hq16.tile([P, 512], bf16, tag="k16", name="k16_t")
                        return (wq_t, wk_t, qt_t, kt_t, q16_t, k16_t)

                    def proj_steps(tiles):
                        wq_t, wk_t, qt_t, kt_t, q16_t, k16_t = tiles
                        steps = []
                        for w_t, dst8, dst16 in ((wq_t, qt_t, q16_t),
                                                 (wk_t, kt_t, k16_t)):
                            for tbp in range(2):
                                def step(w_t=w_t, dst8=dst8, dst16=dst16,
                                         tbp=tbp):
                                    tb0, tb1 = 2 * tbp, 2 * tbp + 1
                                    psa = psA.tile([P, 512], f32, tag="psa",
                                                   name="ps_a0")
                                    psb = psA.tile([P, 512], f32, tag="psa",
                                                   name="ps_a1")
                                    # both tb blocks share each stationary
                                    for pr in range(NPR):
                                        for ps_, tb in ((psa, tb0),
                                                        (psb, tb1)):
                                            nc.tensor.matmul(
                                                ps_,
                                                w_t[:, 2 * pr:2 * pr + 2],
                                                xt[pr][:, :, tb * 512:
                                                       (tb + 1) * 512],
                                                start=(pr == 0),
                                                stop=(pr == NPR - 1),
                                                perf_mode=DR)
                                    with nc.allow_low_precision("fp8 quant"):
                                        for ps_, tb in ((psa, tb0),
                                                        (psb, tb1)):
                                            nc.vector.tensor_scalar_mul(
                                                dst8[:, tb * 512:
                                                     (tb + 1) * 512],
                                                ps_, QSC)
                                        if tbp == 0:
                                            nc.vector.tensor_scalar_mul(
                                                dst16, psa, QSC)
                                steps.append(step)
                        return steps

                    def attn_steps(h, tiles):
                        wq_t, wk_t, qt_t, kt_t, q16_t, k16_t = tiles
                        st = {}
                        steps = []

                        def new_block():
                            st["po"] = psO.tile([P, 512], f32, tag="po", name="po_t")
                            st["rs"] = psO.tile([P, 512], f32, tag="po", name="rs_t")
                            st["pts"] = {}

                        def s16_step(m):
                            q0 = m * P
                            if m % 2 == 0:
                                st["ps16"] = psSp.tile([P, 2, 512], f32, tag="pss", name="ps16_t")
                            ps = st["ps16"]
                            nc.tensor.matmul(
                                ps[:, m % 2, q0:], k16_t[:, m * P:(m + 1) * P],
                                q16_t[:, q0:], start=True, stop=True)
                            if m % 2 == 1:
                                pt = pt16p.tile([P, 2, 512], bf16, tag="pt16", name="pt16_t")
                                for j in (m - 1, m):
                                    jq0 = j * P
                                    nc.scalar.activation(
                                        pt[:, j % 2, jq0:], ps[:, j % 2, jq0:],
                                        EXP, scale=EXP_SCALE)
                                    with nc.allow_low_precision("mask"):
                                        nc.gpsimd.tensor_tensor(
                                            pt[:, j % 2, jq0:jq0 + P],
                                            pt[:, j % 2, jq0:jq0 + P],
                                            tri16_t, MULT)
                                    st["pts"][j] = pt

                        def av16_step(m):
                            q0 = m * P
                            pt = st["pts"].pop(m)
                            nc.tensor.matmul(
                                st["po"][:, q0:], v16_t[:, m, h],
                                pt[:, m % 2, q0:],
                                start=(m == 0), stop=(m == 3))
                            nc.tensor.matmul(
                                st["rs"][:, q0:], ones16_t, pt[:, m % 2, q0:],
                                start=(m == 0), stop=(m == 3))

                        def s8_step(kp, jb):
                            ks0 = 2 * kp
                            diag = ks0 - 4 * jb
                            ps = psSp.tile([P, 2, 512], f32, tag="pss", name="ps8_t")
                            pt = pt8p.tile([P, 2, 512], f8, tag="pt8", name="pt8_t")
                            if diag < 0:          # full pair
                                for j in range(2):
                                    nc.tensor.matmul(
                                        ps[:, j],
                                        kt_t[:, (ks0 + j) * P:
                                             (ks0 + j + 1) * P],
                                        qt_t[:, jb * 512:(jb + 1) * 512],
                                        start=True, stop=True)
                                nc.scalar.activation(
                                    pt, ps, EXP, scale=EXP_SCALE)
                            else:
                                m0 = diag      # 0 or 2
                                q00, q01 = m0 * P, (m0 + 1) * P
                                for j in range(2):
                                    nc.tensor.matmul(
                                        ps[:, j, q00:],
                                        kt_t[:, (ks0 + j) * P:
                                             (ks0 + j + 1) * P],
                                        qt_t[:, jb * 512 + q00:
                                             (jb + 1) * 512],
                                        start=True, stop=True)
                                nc.scalar.activation(
                                    pt[:, :, q00:], ps[:, :, q00:],
                                    EXP, scale=EXP_SCALE)
                                nc.gpsimd.memset(pt[:, 1, q00:q01], 0.0)
                                with nc.allow_low_precision("mask"):
                                    nc.gpsimd.tensor_tensor(
                                        pt[:, 0, q00:q01],
                                        pt[:, 0, q00:q01],
                                        tri8_t, MULT)
                                    nc.gpsimd.tensor_tensor(
                                        pt[:, 1, q01:q01 + P],
                                        pt[:, 1, q01:q01 + P],
                                        tri8_t, MULT)
                            st["pts"][kp] = (pt, max(0, diag) * P)

                        def av8_step(kp, npair):
                            pt, q0 = st["pts"][kp]
                            nc.tensor.matmul(
                                st["po"][:, q0:],
                                v8_t[:, 2 * kp:2 * kp + 2, h, :],
                                pt[:, :, q0:],
                                start=(kp == 0), stop=(kp == npair - 1),
                                perf_mode=DR)

                        def rs8_burst(npair):
                            # all rowsums back-to-back: constant ones8
                            # stationary is reloaded only once
                            for kp in range(npair):
                                pt, q0 = st["pts"].pop(kp)
                                nc.tensor.matmul(
                                    st["rs"][:, q0:], ones8_t, pt[:, :, q0:],
                                    start=(kp == 0), stop=(kp == npair - 1),
                                    perf_mode=DR)

                        def norm(jb):
                            po_, rs_ = st["po"], st["rs"]
                            recip = rcp.tile([P, 512], f32, tag="rc", name="recip_t")
                            with nc.allow_low_precision("recip"):
                                nc.vector.reciprocal_approx_fast(recip, rs_)
                            with nc.allow_low_precision("fp8 quant"):
                                if jb == 0:
                                    nc.vector.tensor_tensor(
                                        ot16_t[:, h], po_[:, 0:P],
                                        recip[:, 0:P], MULT)
                                    nc.vector.tensor_tensor(
                                        ot8_t[:, h, P:512], po_[:, P:],
                                        recip[:, P:], MULT)
                                else:
                                    nc.vector.tensor_tensor(
                                        ot8_t[:, h, jb * 512:(jb + 1) * 512],
                                        po_, recip, MULT)

                        # jb0 bf16 path, software-pipelined
                        steps.append((lambda: (new_block(), s16_step(0),
                                               s16_step(1)), None))
                        steps.append((lambda: (s16_step(2), av16_step(0)),
                                      None))
                        steps.append((lambda: (s16_step(3), av16_step(1)),
                                      None))
                        steps.append((lambda: (av16_step(2), av16_step(3)),
                                      None))
                        steps.append((lambda: norm(0), 0))
                        # fp8 jbs
                        for jb in range(1, NTB):
                            npair = 2 * (jb + 1)
                            steps.append((lambda jb=jb: (new_block(),
                                                         s8_step(0, jb)),
                                          None))
                            for kp in range(npair):
                                def step(kp=kp, jb=jb, npair=npair):
                                    if kp + 1 < npair:
                                        s8_step(kp + 1, jb)
                                    av8_step(kp, npair)
                                steps.append((step, None))
                            steps.append((lambda npair=npair:
                                          rs8_burst(npair), None))
                            steps.append((lambda jb=jb: norm(jb), jb))
                        return steps

                    # ---------------- V projection (all heads) ----------------
                    with tc.tile_pool(name="wv16p", bufs=1) as wv16p, \
                         tc.tile_pool(name="psV", bufs=2, space="PSUM") as psV:
                        wv16_t = wv16p.tile([P, NCS, HG * D], bf16, tag="wv16")
                        nc.sync.dma_start(wv16_t, wv16_d[:])
                        tiles0 = load_head(0)
                        p0 = proj_steps(tiles0)
                        pi0 = 0
                        for idx, tch in enumerate(list(range(1, NTC)) + [0]):
                            ps = psV.tile([P, 2, 512], f32, tag="psv")
                            for half in range(2):
                                if tch == 0:
                                    # bf16 V proj for k<128 (accuracy)
                                    for cs in range(NCS):
                                        nc.tensor.matmul(
                                            ps[:, half],
                                            xt16_t[:, cs],
                                            wv16_t[:, cs,
                                                   half * 512:(half + 1) * 512],
                                            start=(cs == 0), stop=(cs == NCS - 1))
                                else:
                                    for pr in range(NPR):
                                        nc.tensor.matmul(
                                            ps[:, half],
                                            xt[pr][:, :, tch * P:(tch + 1) * P],
                                            wv_t[:, 2 * pr:2 * pr + 2,
                                                 half * 512:(half + 1) * 512],
                                            start=(pr == 0), stop=(pr == NPR - 1),
                                            perf_mode=DR)
                            with nc.allow_low_precision("fp8 quant"):
                                nc.vector.tensor_scalar_mul(
                                    v8_t[:, tch], ps, QSC)
                                if tch < 4:
                                    nc.vector.tensor_scalar_mul(
                                        v16_t[:, tch], ps, QSC)
                            want0 = (idx + 1) * len(p0) // NTC
                            while pi0 < want0:
                                p0[pi0]()
                                pi0 += 1

                    with tc.tile_pool(name="psS", bufs=2,
                                      space="PSUM") as psSp:
                        tiles = tiles0
                        for h in range(HG):
                            asteps = attn_steps(h, tiles)
                            if h + 1 < HG:
                                tiles = load_head(h + 1)
                                psteps = proj_steps(tiles)
                            else:
                                psteps = []
                            na, pi = len(asteps), 0
                            for i, (s, mark) in enumerate(asteps):
                                s()
                                if h == HG - 1 and mark is not None \
                                        and mark < NTB - 1:
                                    # rows of block `mark` are final: run
                                    # their fp8 out-proj now (tch0 is bf16,
                                    # handled in the tail)
                                    t0 = 4 * mark if mark else 1
                                    for tch in range(t0, 4 * mark + 4):
                                        for obp in range(2):
                                            c_fp8_step(tch, obp)
                                want = (i + 1) * len(psteps) // na
                                while pi < want:
                                    psteps[pi]()
                                    pi += 1

                    # ---------------- output projection ----------------
                    with tc.tile_pool(name="wo16p", bufs=4) as wo16p, \
                         tc.tile_pool(name="psC", bufs=4, space="PSUM") as psC:
                        # ob pairs share each stationary (halves LDW reloads)
                        for obp in range(2):
                            wo16_a = wo16p.tile([P, HG, 512], bf16,
                                                tag="wo16", name="wo16_a")
                            nc.sync.dma_start(
                                wo16_a,
                                wo16_d[:, :, (2 * obp) * 512:
                                       (2 * obp + 1) * 512])
                            wo16_b = wo16p.tile([P, HG, 512], bf16,
                                                tag="wo16", name="wo16_b")
                            nc.sync.dma_start(
                                wo16_b,
                                wo16_d[:, :, (2 * obp + 1) * 512:
                                       (2 * obp + 2) * 512])
                            for tch in [0] + list(range(12, NTC)):
                                ps2 = [psC.tile([P, 512], f32, tag="psc",
                                                name=f"psc{j}")
                                       for j in range(2)]
                                if tch == 0:
                                    for hh in range(HG):
                                        for j, wot in enumerate(
                                                (wo16_a, wo16_b)):
                                            nc.tensor.matmul(
                                                ps2[j], ot16_t[:, hh],
                                                wot[:, hh],
                                                start=(hh == 0),
                                                stop=(hh == HG - 1))
                                else:
                                    for g2 in range(HG // 2):
                                        for j in range(2):
                                            ob = 2 * obp + j
                                            nc.tensor.matmul(
                                                ps2[j],
                                                ot8_t[:, 2 * g2:2 * g2 + 2,
                                                      tch * P:(tch + 1) * P],
                                                wo_t[:, 2 * g2:2 * g2 + 2,
                                                     ob * 512:(ob + 1) * 512],
                                                start=(g2 == 0),
                                                stop=(g2 == HG // 2 - 1),
                                                perf_mode=DR)
                                for j in range(2):
                                    ob = 2 * obp + j
                                    st = cstage.tile([P, 512], f32, tag="cst",
                                                     name="cst_t")
                                    if j == 0:
                                        nc.scalar.activation(st, ps2[j], COPY,
                                                             scale=OSC)
                                    else:
                                        with nc.allow_low_precision("osc"):
                                            nc.vector.tensor_scalar_mul(
                                                st, ps2[j], OSC)
                                    nc.sync.dma_start(
                                        out_d[tch * P:(tch + 1) * P,
                                              ob * 512:(ob + 1) * 512], st)

    nc.finalize()
    return nc


VMODE = "fp8"


def _get_nc():
    global _CACHED
    if _CACHED is None:
        _CACHED = _build()
    return _CACHED


def _prep_inputs(x, W_qkv, W_out, vmode=None):
    """Host-side shard + quantize + layout prep. Returns per-core input maps."""
    import ml_dtypes
    E4 = ml_dtypes.float8_e4m3
    BF = ml_dtypes.bfloat16
    f32 = np.float32
    x = np.asarray(x, dtype=f32)
    W_qkv = np.asarray(W_qkv, dtype=f32)
    W_out = np.asarray(W_out, dtype=f32)

    k_idx = np.arange(P)
    tri = (np.arange(P)[None, :] >= k_idx[:, None]).astype(f32)  # [k, q]
    tri8 = tri.astype(E4)
    tri16 = tri.astype(BF)
    ones8 = np.full((P, 2, P), 1.0 / 16.0, dtype=E4)
    ones16 = np.full((P, P), 1.0 / 16.0, dtype=BF)

    per_g = {}
    for g in range(2):
        sl = slice(g * HG * D, (g + 1) * HG * D)
        wq = (W_qkv[0 * C:1 * C][sl] * 512.0)
        wk = (W_qkv[1 * C:2 * C][sl] * 512.0)
        wv = (W_qkv[2 * C:3 * C][sl] * 512.0)
        # [h, p, cs, m]: element = w[h*128+m, cs*128+p]
        wq_a = np.ascontiguousarray(
            wq.reshape(HG, D, NCS, P).transpose(0, 3, 2, 1)).astype(E4)
        wk_a = np.ascontiguousarray(
            wk.reshape(HG, D, NCS, P).transpose(0, 3, 2, 1)).astype(E4)
        # [p, cs, hm]: element = wv[hm, cs*128+p]
        wv_r = np.ascontiguousarray(
            wv.reshape(HG * D, NCS, P).transpose(2, 1, 0))
        wv_a = wv_r.astype(E4)
        wv16_a = wv_r.astype(BF)
        # [p(d), h, o]: element = W_out[o, g*1024 + h*128 + d] * 512
        wo_r = np.ascontiguousarray(
            (W_out[:, sl] * 512.0).reshape(C, HG, D).transpose(2, 1, 0))
        wo_a = wo_r.astype(E4)
        wo16_a = wo_r.astype(BF)
        per_g[g] = (wq_a, wk_a, wv_a, wv16_a, wo_a, wo16_a)

    in_maps = []
    for core in range(8):
        b, g = divmod(core, 2)
        xT = np.ascontiguousarray(x[b].T)                      # [C, T]
        xt8 = xT.reshape(NPR, 2, P, T).transpose(0, 2, 1, 3)   # [pr, p, 2, T]
        xt8 = np.ascontiguousarray(xt8).astype(E4)
        xt16 = np.ascontiguousarray(
            xT[:, :P].reshape(NCS, P, P).transpose(1, 0, 2)).astype(BF)
        wq_a, wk_a, wv_a, wv16_a, wo_a, wo16_a = per_g[g]
        im = {
            "xt": xt8, "xt16": xt16, "wq": wq_a, "wk": wk_a,
            "wv": wv_a, "wv16": wv16_a, "wo": wo_a, "wo16": wo16_a,
            "tri8": tri8, "tri16": tri16, "ones8": ones8, "ones16": ones16,
        }
        in_maps.append(im)
    return in_maps


def kernel(x, W_qkv, W_out, *, trace=False, trace_cores=None):
    from concourse.bass_utils import run_bass_kernel_spmd

    nc = _get_nc()
    in_maps = _prep_inputs(x, W_qkv, W_out)
    r = run_bass_kernel_spmd(
        nc, in_maps, core_ids=list(range(8)),
        trace=trace, trace_cores=trace_cores)

    out = np.empty((B, T, C), dtype=np.float32)
    for b in range(B):
        out[b] = r.results[2 * b]["out"] + r.results[2 * b + 1]["out"]
    if trace:
        kernel.last_results = r
    return out
